# revision 11
# baseline (speedup 1.0000x reference)
"""EdgeConv + GroupNorm fused Trainium2 kernel (8 NeuronCores).

Reference computation (per batch b):
    local = w1 @ feature[b]            # (128, N)
    edge  = w2 @ feature[b]            # (128, N)
    nb[c,n,j]   = edge[c, idx[n,j]]
    ef = concat([local bcast K, nb - local], ch) -> (256, N, K)
    ef = GroupNorm(4 groups over 256 ch, stats over (ch_grp, N, K))
    out = mean_j relu(ef)              # (256, N)

Sharding: core i handles batch b=i//4, N-chunk q=i%4 (4096 points).
Each core computes the full per-batch edge matrix (replicated small matmul),
writes it transposed+bf16 to local DRAM, then uses the SWDGE transpose-gather
(dma_gather) to pull its chunk's 65536 neighbour columns with channels landing
on partitions.  GroupNorm statistics are partial per core and combined with a
tiny AllReduce over each batch's 4 cores.
"""

import numpy as np
import ml_dtypes
from contextlib import ExitStack

import concourse.bass as bass
import concourse.bacc as bacc
import concourse.tile as tile
from concourse import mybir
from concourse.bass_utils import run_bass_kernel_spmd

B = 2
C = 128          # in/out channels per conv
N = 16384        # points
K = 16           # neighbours
NCORES = 8
CORES_PER_BATCH = 4
SHARD_N = N // CORES_PER_BATCH          # 4096 points per core
CHUNK_N = 512                            # points per gather chunk
NCHUNKS = SHARD_N // CHUNK_N             # 8
IDX_PER_CHUNK = CHUNK_N * K              # 8192
EPS = 1e-5
CNT_CEN = 64 * N                         # stats count for central groups
CNT_DIF = 64 * N * K                     # stats count for diff groups

F32 = mybir.dt.float32
BF16 = mybir.dt.bfloat16
I16 = mybir.dt.int16
AX = mybir.AxisListType
ALU = mybir.AluOpType
ACTF = mybir.ActivationFunctionType


def build_body(ctx, tc, featT, featC, w1T, w2T, idx16, gamma, beta, out, phase=99):
    # phase: 1=matmuls only, 2=+gather, 3=+stats/fold/collective, 4=+bcast/params,
    #        99=full
    nc = tc.nc

    singles = ctx.enter_context(tc.tile_pool(name="singles", bufs=1))
    feat_pool = ctx.enter_context(tc.tile_pool(name="feat", bufs=3))
    stage_pool = ctx.enter_context(tc.tile_pool(name="stage", bufs=3))
    g_pool = ctx.enter_context(tc.tile_pool(name="g", bufs=NCHUNKS))
    sq_pool = ctx.enter_context(tc.tile_pool(name="sq", bufs=2))
    y_pool = ctx.enter_context(tc.tile_pool(name="y", bufs=2))
    oc_pool = ctx.enter_context(tc.tile_pool(name="oc", bufs=2))
    od_pool = ctx.enter_context(tc.tile_pool(name="od", bufs=2))
    psum_mm = ctx.enter_context(tc.tile_pool(name="psmm", bufs=2, space="PSUM"))
    psum_sm = ctx.enter_context(tc.tile_pool(name="pssm", bufs=2, space="PSUM"))
    dram = ctx.enter_context(tc.tile_pool(name="dram", bufs=1, space="DRAM"))

    # ---------------- constants ----------------
    w1t_sb = singles.tile([C, C], BF16)
    nc.sync.dma_start(out=w1t_sb, in_=w1T)
    w2t_sb = singles.tile([C, C], BF16)
    nc.sync.dma_start(out=w2t_sb, in_=w2T)
    idx_sb = singles.tile([128, SHARD_N], I16)
    nc.sync.dma_start(out=idx_sb, in_=idx16)
    gam_c = singles.tile([C, 1], F32)
    nc.sync.dma_start(out=gam_c, in_=gamma[0:C].rearrange("(p o) -> p o", o=1))
    gam_d = singles.tile([C, 1], F32)
    nc.sync.dma_start(out=gam_d, in_=gamma[C : 2 * C].rearrange("(p o) -> p o", o=1))
    bet_c = singles.tile([C, 1], F32)
    nc.sync.dma_start(out=bet_c, in_=beta[0:C].rearrange("(p o) -> p o", o=1))
    bet_d = singles.tile([C, 1], F32)
    nc.sync.dma_start(out=bet_d, in_=beta[C : 2 * C].rearrange("(p o) -> p o", o=1))

    # ---------------- edgeT = (w2 @ feature).T in DRAM, bf16 ----------------
    edgeT = dram.tile([N, C], BF16)
    for t in range(N // 512):          # 32 tiles of 512 points
        ft = feat_pool.tile([C, 512], BF16)
        nc.sync.dma_start(out=ft, in_=featT[:, t * 512 : (t + 1) * 512])
        ps = psum_mm.tile([128, 4, C], F32)
        for q in range(4):
            # out[n_block, c] = sum_k feature[k, n] * w2T[k, c]
            nc.tensor.matmul(
                ps[:, q, :],
                lhsT=ft[:, q * 128 : (q + 1) * 128],
                rhs=w2t_sb,
                start=True,
                stop=True,
            )
        st = stage_pool.tile([128, 4, C], BF16)
        nc.any.tensor_copy(out=st, in_=ps)
        # rows n = (4t+q)*128 + p
        dst = edgeT[t * 512 : (t + 1) * 512, :].rearrange("(q p) c -> p q c", p=128)
        nc.sync.dma_start(out=dst, in_=st)

    # ---------------- local = w1 @ feature[:, chunk], bf16 in SBUF ----------
    l_bf = singles.tile([C, SHARD_N], BF16)
    for t in range(SHARD_N // 512):    # 8 tiles
        fc = feat_pool.tile([C, 512], BF16)
        nc.sync.dma_start(out=fc, in_=featC[:, t * 512 : (t + 1) * 512])
        psl = psum_mm.tile([128, 512], F32)
        nc.tensor.matmul(psl, lhsT=w1t_sb, rhs=fc, start=True, stop=True)
        nc.any.tensor_copy(out=l_bf[:, t * 512 : (t + 1) * 512], in_=psl)

    if phase < 2:
        return
    # ---------------- gather + diff stats ----------------
    stat_d = singles.tile([128, NCHUNKS], F32)     # per-chunk sum(D)
    stat_d2 = singles.tile([128, NCHUNKS * 4], F32)  # per-quarter sum(D^2)
    g_tiles = []
    for k in range(NCHUNKS):
        g = g_pool.tile([128, 1, IDX_PER_CHUNK], BF16)
        nc.gpsimd.dma_gather(
            out_ap=g[:, :, :],
            in_ap=edgeT[:, :],
            idxs_ap=idx_sb[:, k * CHUNK_N : (k + 1) * CHUNK_N],
            num_idxs=IDX_PER_CHUNK,
            num_idxs_reg=IDX_PER_CHUNK,
            elem_size=C,
            transpose=True,
            single_packet=False,
        )
        g_tiles.append(g)

        if phase < 21:
            continue
        # D = G - local (in place over G), per j-row
        gv = g[:, 0, :].rearrange("p (j n) -> p j n", j=K)        # [128, 16, 512]
        lb = l_bf[:, k * CHUNK_N : (k + 1) * CHUNK_N]             # [128, 512]
        for j in range(K):
            nc.vector.tensor_tensor(
                out=gv[:, j, :], in0=gv[:, j, :], in1=lb, op=ALU.subtract
            )
        gflat = g[:, 0, :]
        nc.vector.tensor_reduce(
            out=stat_d[:, k : k + 1], in_=gflat, axis=AX.X, op=ALU.add
        )
        # sum(D^2) via ACT square with free-dim accumulate (quarter pieces)
        if phase < 22:
            continue
        for p4 in range(4):
            sq = sq_pool.tile([128, 2048], BF16)
            nc.scalar.activation(
                out=sq,
                in_=gflat[:, p4 * 2048 : (p4 + 1) * 2048],
                func=ACTF.Square,
                accum_out=stat_d2[:, k * 4 + p4 : k * 4 + p4 + 1],
            )

    if phase < 25:
        return
    # ---------------- central (local) stats ----------------
    stats4 = singles.tile([128, 4], F32)   # cols: sumL, sumD, sumL2, sumD2
    nc.vector.tensor_reduce(
        out=stats4[:, 0:1], in_=l_bf, axis=AX.X, op=ALU.add
    )
    stat_l2 = singles.tile([128, 2], F32)
    for h in range(2):
        sql = sq_pool.tile([128, 2048], BF16, tag="sq")
        nc.scalar.activation(
            out=sql,
            in_=l_bf[:, h * 2048 : (h + 1) * 2048],
            func=ACTF.Square,
            accum_out=stat_l2[:, h : h + 1],
        )
    nc.vector.tensor_reduce(out=stats4[:, 2:3], in_=stat_l2, axis=AX.X, op=ALU.add)
    nc.vector.tensor_reduce(out=stats4[:, 1:2], in_=stat_d, axis=AX.X, op=ALU.add)
    nc.vector.tensor_reduce(out=stats4[:, 3:4], in_=stat_d2, axis=AX.X, op=ALU.add)

    # ---------------- fold across partitions (64-partition halves) ----------
    mask2 = singles.tile([128, 2], F32)
    nc.vector.memset(mask2, 0.0)
    nc.vector.memset(mask2[0:64, 0:1], 1.0)
    nc.vector.memset(mask2[64:128, 1:2], 1.0)
    fold_ps = psum_sm.tile([4, 2], F32)
    nc.tensor.matmul(fold_ps, lhsT=stats4, rhs=mask2, start=True, stop=True)
    fold_sb = singles.tile([4, 2], F32)
    nc.vector.tensor_copy(out=fold_sb, in_=fold_ps)

    if phase < 30:
        return
    # ---------------- AllReduce partial stats within each batch group ------
    cc_in = dram.tile([8], F32)
    cc_out = dram.tile([8], F32)
    nc.gpsimd.dma_start(out=cc_in.rearrange("(p f) -> p f", f=2), in_=fold_sb)
    nc.gpsimd.collective_compute(
        "AllReduce",
        ALU.add,
        replica_groups=[[0, 1, 2, 3], [4, 5, 6, 7]],
        ins=[cc_in.opt()],
        outs=[cc_out.opt()],
    )
    # row8 = [S_g0 S_g1 S_g2 S_g3 | Q_g0 Q_g1 Q_g2 Q_g3]
    row8 = singles.tile([1, 8], F32)
    nc.sync.dma_start(out=row8, in_=cc_out.rearrange("(o f) -> o f", o=1))

    if phase < 40:
        return
    # ---------------- group mean / rstd ----------------
    inv4 = singles.tile([1, 4], F32)
    nc.vector.memset(inv4[:, 0:2], 1.0 / CNT_CEN)
    nc.vector.memset(inv4[:, 2:4], 1.0 / CNT_DIF)
    mean4 = singles.tile([1, 4], F32)
    nc.vector.tensor_tensor(out=mean4, in0=row8[:, 0:4], in1=inv4, op=ALU.mult)
    esq4 = singles.tile([1, 4], F32)
    nc.vector.tensor_tensor(out=esq4, in0=row8[:, 4:8], in1=inv4, op=ALU.mult)
    m2 = singles.tile([1, 4], F32)
    nc.vector.tensor_tensor(out=m2, in0=mean4, in1=mean4, op=ALU.mult)
    var4 = singles.tile([1, 4], F32)
    nc.vector.tensor_tensor(out=var4, in0=esq4, in1=m2, op=ALU.subtract)
    eps1 = singles.tile([1, 1], F32)
    nc.vector.memset(eps1, EPS)
    rs4 = singles.tile([1, 4], F32)
    nc.scalar.activation(out=rs4, in_=var4, func=ACTF.Sqrt, bias=eps1, scale=1.0)
    nc.vector.reciprocal(out=rs4, in_=rs4)

    mrow8 = singles.tile([1, 8], F32)
    nc.vector.tensor_copy(out=mrow8[:, 0:4], in_=mean4)
    nc.vector.tensor_copy(out=mrow8[:, 4:8], in_=rs4)
    bc8 = singles.tile([128, 8], F32)
    nc.gpsimd.partition_broadcast(bc8, mrow8)

    def half_select(dst, col0, col1):
        nc.vector.tensor_copy(out=dst[0:64, :], in_=bc8[0:64, col0 : col0 + 1])
        nc.vector.tensor_copy(out=dst[64:128, :], in_=bc8[64:128, col1 : col1 + 1])

    mean_c = singles.tile([128, 1], F32)
    half_select(mean_c, 0, 1)
    mean_d = singles.tile([128, 1], F32)
    half_select(mean_d, 2, 3)
    rs_c = singles.tile([128, 1], F32)
    half_select(rs_c, 4, 5)
    rs_d = singles.tile([128, 1], F32)
    half_select(rs_d, 6, 7)

    # per-partition affine params
    s_cen = singles.tile([128, 1], F32)
    nc.vector.tensor_tensor(out=s_cen, in0=gam_c, in1=rs_c, op=ALU.mult)
    t0 = singles.tile([128, 1], F32)
    nc.vector.tensor_tensor(out=t0, in0=mean_c, in1=s_cen, op=ALU.mult)
    b_cen = singles.tile([128, 1], F32)
    nc.vector.tensor_tensor(out=b_cen, in0=bet_c, in1=t0, op=ALU.subtract)

    s_dif = singles.tile([128, 1], F32)
    nc.vector.tensor_tensor(out=s_dif, in0=gam_d, in1=rs_d, op=ALU.mult)
    nc.vector.tensor_scalar_mul(s_dif, s_dif, 1.0 / K)
    t1 = singles.tile([128, 1], F32)
    nc.vector.tensor_tensor(out=t1, in0=mean_d, in1=s_dif, op=ALU.mult)
    t2 = singles.tile([128, 1], F32)
    nc.vector.tensor_scalar_mul(t2, bet_d, 1.0 / K)
    b_dif = singles.tile([128, 1], F32)
    nc.vector.tensor_tensor(out=b_dif, in0=t2, in1=t1, op=ALU.subtract)

    if phase < 50:
        return
    # ---------------- central half apply ----------------
    for t in range(SHARD_N // 1024):   # 4 pieces
        oc = oc_pool.tile([128, 1024], F32)
        nc.scalar.activation(
            out=oc,
            in_=l_bf[:, t * 1024 : (t + 1) * 1024],
            func=ACTF.Relu,
            bias=b_cen,
            scale=s_cen,
        )
        nc.sync.dma_start(out=out[0:C, t * 1024 : (t + 1) * 1024], in_=oc)

    # ---------------- diff half apply: relu(s*D+b), K-mean via add tree ----
    for k in range(NCHUNKS):
        dv = g_tiles[k][:, 0, :].rearrange("p (j n) -> p j n", j=K)  # [128,16,512]
        for h in range(2):
            piece = dv[:, :, h * 256 : (h + 1) * 256]                # [128,16,256]
            ya = y_pool.tile([128, K, 256], BF16)
            nc.scalar.activation(
                out=ya, in_=piece, func=ACTF.Relu, bias=b_dif, scale=s_dif
            )
            nc.vector.tensor_tensor(
                out=ya[:, 0:8, :], in0=ya[:, 0:8, :], in1=ya[:, 8:16, :], op=ALU.add
            )
            nc.vector.tensor_tensor(
                out=ya[:, 0:4, :], in0=ya[:, 0:4, :], in1=ya[:, 4:8, :], op=ALU.add
            )
            nc.vector.tensor_tensor(
                out=ya[:, 0:2, :], in0=ya[:, 0:2, :], in1=ya[:, 2:4, :], op=ALU.add
            )
            od = od_pool.tile([128, 256], F32)
            nc.vector.tensor_tensor(
                out=od, in0=ya[:, 0, :], in1=ya[:, 1, :], op=ALU.add
            )
            col = k * CHUNK_N + h * 256
            nc.sync.dma_start(out=out[C : 2 * C, col : col + 256], in_=od)


def build_nc(phase=99):
    nc = bacc.Bacc(
        "TRN2", target_bir_lowering=False, debug=False, num_devices=NCORES
    )
    featT = nc.dram_tensor("feat", [C, N], BF16, kind="ExternalInput").ap()
    featC = nc.dram_tensor("featc", [C, SHARD_N], BF16, kind="ExternalInput").ap()
    w1T = nc.dram_tensor("w1t", [C, C], BF16, kind="ExternalInput").ap()
    w2T = nc.dram_tensor("w2t", [C, C], BF16, kind="ExternalInput").ap()
    idx16 = nc.dram_tensor("idx16", [128, SHARD_N], I16, kind="ExternalInput").ap()
    gamma = nc.dram_tensor("gamma", [2 * C], F32, kind="ExternalInput").ap()
    beta = nc.dram_tensor("beta", [2 * C], F32, kind="ExternalInput").ap()
    out = nc.dram_tensor("out", [2 * C, SHARD_N], F32, kind="ExternalOutput").ap()

    with tile.TileContext(nc) as tc:
        with ExitStack() as ctx:
            build_body(ctx, tc, featT, featC, w1T, w2T, idx16, gamma, beta, out,
                       phase=phase)
    nc.compile()
    return nc


_nc_cache = None


def _get_nc():
    global _nc_cache
    if _nc_cache is None:
        _nc_cache = build_nc()
    return _nc_cache


def make_in_maps(feature, knn_inds, w1, w2, gamma, beta):
    bf16 = ml_dtypes.bfloat16
    feature = np.asarray(feature)
    knn_inds = np.asarray(knn_inds)
    w1t = np.ascontiguousarray(np.asarray(w1).T).astype(bf16)
    w2t = np.ascontiguousarray(np.asarray(w2).T).astype(bf16)
    gamma = np.ascontiguousarray(np.asarray(gamma)).astype(np.float32)
    beta = np.ascontiguousarray(np.asarray(beta)).astype(np.float32)
    in_maps = []
    for core in range(NCORES):
        b, q = divmod(core, CORES_PER_BATCH)
        n0 = q * SHARD_N
        featb = np.ascontiguousarray(feature[b]).astype(bf16)
        featc = np.ascontiguousarray(feature[b][:, n0 : n0 + SHARD_N]).astype(bf16)
        idx = knn_inds[b, n0 : n0 + SHARD_N, :].astype(np.int64)
        # j-major within each 512-point chunk: pos = k*8192 + j*512 + n_loc
        idxl = (
            idx.reshape(NCHUNKS, CHUNK_N, K).transpose(0, 2, 1).reshape(-1)
        )
        wrapped = idxl.reshape(-1, 16).T.astype(np.int16)   # [16, SHARD_N]
        idx16 = np.ascontiguousarray(np.tile(wrapped, (8, 1)))  # [128, SHARD_N]
        in_maps.append(
            {
                "feat": featb,
                "featc": featc,
                "w1t": w1t,
                "w2t": w2t,
                "idx16": idx16,
                "gamma": gamma,
                "beta": beta,
            }
        )
    return in_maps


def assemble_output(results):
    out = np.zeros((B, 2 * C, N), np.float32)
    for core in range(NCORES):
        b, q = divmod(core, CORES_PER_BATCH)
        out[b, :, q * SHARD_N : (q + 1) * SHARD_N] = results[core]["out"]
    return out


def kernel(feature, knn_inds, w1, w2, gamma, beta):
    nc = _get_nc()
    in_maps = make_in_maps(feature, knn_inds, w1, w2, gamma, beta)
    res = run_bass_kernel_spmd(nc, in_maps, core_ids=list(range(NCORES)))
    return assemble_output(res.results)
